# revision 4
# baseline (speedup 1.0000x reference)
"""Trainium2 Bass kernel for nn_Classifier_custom_12936441496172.

Reference math (per batch b, with av = column-l2-normalized img_b [Cf, R]):
    A      = softmax_r( (vv @ W1) @ av )          # [I, R] attention over R
    F_p    = A @ av.T                             # [I, Cf]
    out[b] = rowsum( (vv @ W2) * F_p )            # [I]

Key identity: out[b, i] = sum_r A[i, r] * ((vv @ W2) @ av)[i, r], so F_p is
never materialized. (vv@W1)@av and (vv@W2)@av come from one stacked weight
matrix qpt, and the column normalization of av folds into a per-column scale
rn[r] = 1/||img_b[:, r]|| applied to the matmul outputs.

Sharding: data-parallel over batch across 8 NeuronCores (16 batches each),
parameters replicated. Parameter prep (vv @ W1/W2, < 1% of FLOPs) on host.

Device kernel per core: 8 groups of 2 batches (N = 512 matmul free dim).
  - img arrives as one [128, 4096] bf16 tile per group (two dma_starts with
    4 KB per-partition lines), so the DMA issue path is 2 descriptors-lists
    per group instead of 8 and the ramp is bandwidth- not issue-limited.
  - norms: squares and a 3-level add tree run at [128, 1024-2048] grain
    split across DVE/ACT/GpSimd, then gpsimd partition_all_reduce produces
    n2 broadcast on all 128 partitions (no TensorE ones-matmuls, no
    separate broadcast); rn = Exp(-0.5 * Ln(n2)) on ACT.
  - main: 5 m-chunks of the 624 stacked rows (tail packs Q/P 56-row
    remainders), each 8 accumulating bf16 matmuls; the PE runs nothing else.
  - drains per chunk-pair: sqs = qa*rn (DVE, reads PSUM), E = Exp(sqs) with
    free-axis accum -> sumexp columns (ACT), sps = pa*rn, then a fused
    scalar_tensor_tensor E*sps with accum -> unnormalized output column.
  - softmax denominators are applied per PAIR of groups (reciprocal +
    multiply on [*, 4] tiles + output DMA), so the kernel tail is only the
    last group's drain chain.
All ACT functions (Square/Ln/Exp) are pinned to the one act-func set that
contains all three, so the ACT engine never reloads activation tables
mid-kernel. PE warm-up: ~14 dummy matmuls on a gpsimd-memset tile while the
first DMAs are in flight. Logits are ~N(0,1) so softmax max-subtraction is
skipped; exp cannot overflow fp32.
"""

import numpy as np

_PROGRAM = None

# Problem geometry (hardcoded per contract; kernel.py must be self-contained)
N_CORES = 8
NB = 16          # batches per core
R = 256          # H * W
CF = 1024        # feature channels
KC = CF // 128   # 8 contraction chunks
I = 312          # attributes
G = NB // 2      # groups of 2 batches
N = 2 * R        # matmul moving free dim (2 batches)
TQ = I - 256     # 56-row tails
XW = KC * N      # x tile width (4096)
# m-chunk column offsets in the host-reordered qpt
MCH_Q = [0, 128]       # Q rows 0:128, 128:256
MCH_P = [256, 384]     # P rows 0:128, 128:256
MCH_T = 512            # Q rows 256:312 at cols 512:568, P rows at 568:624
N_WARM = 14


def _pin_act_tables(arch):
    """Blank every act-func set except natural_log_exp_and_others (which
    contains Square, Ln and Exp) in the cached table dict, so the table-load
    pass assigns all our activations to that one set and the ACT engine
    performs a single table load instead of flipping per Ln/Exp pair.
    Indices (insertion order) are preserved, so the emitted set id still
    refers to the same canonical act_info.json entry."""
    from concourse.hw_specs import get_activation_tables

    tabs = get_activation_tables(arch)
    target = "natural_log_exp_and_others"
    if target in tabs:
        for name in tabs:
            if name != target:
                tabs[name] = set()


def _build_program():
    import concourse.tile as tile
    from concourse import bacc, bass_isa, mybir

    F32 = mybir.dt.float32
    BF16 = mybir.dt.bfloat16
    MULT = mybir.AluOpType.mult
    EXP = mybir.ActivationFunctionType.Exp
    LN = mybir.ActivationFunctionType.Ln

    nc = bacc.Bacc(
        "TRN2",
        target_bir_lowering=False,
        debug=False,
        enable_asserts=False,
        num_devices=N_CORES,
    )
    _pin_act_tables(nc.m.arch)

    img = nc.dram_tensor("img", [G, 128, XW], BF16, kind="ExternalInput").ap()
    qpt = nc.dram_tensor("qpt", [CF, 2 * I], BF16, kind="ExternalInput").ap()
    out = nc.dram_tensor("out", [I, NB], F32, kind="ExternalOutput").ap()

    with tile.TileContext(nc) as tc, tc.tile_pool(name="sb", bufs=2) as sb, tc.tile_pool(
        name="ps", bufs=7, space="PSUM"
    ) as ps:
        ones_col = nc.const_aps.tensor(1.0, (128, 1), BF16)

        # --- DMA: qpt (8 transfers, 1.25 KB lines), then x groups in order
        # (two [128, 2048] transfers each, 4 KB lines).
        qpt_sb = sb.tile([128, KC * 2 * I], BF16, tag="qpt", bufs=1, name="qpt_sb")
        for k in range(KC):
            nc.sync.dma_start(
                qpt_sb[:, k * 2 * I : (k + 1) * 2 * I], qpt[k * 128 : (k + 1) * 128, :]
            )
        xs = {}
        for g in range(G):
            x = sb.tile([128, XW], BF16, tag="x", bufs=3, name=f"x{g}")
            hw = XW // 2
            nc.sync.dma_start(x[:, :hw], img[g, :, :hw])
            nc.sync.dma_start(x[:, hw:], img[g, :, hw:])
            xs[g] = x

        # Persistent per-core accumulators: unnormalized dots + sumexp matrix.
        MSZ = [128, 128, TQ]
        outsb = [
            sb.tile([msz, NB], F32, tag=f"out{mi}", bufs=1, name=f"outsb{mi}")
            for mi, msz in enumerate(MSZ)
        ]
        semat = [
            sb.tile([msz, NB], F32, tag=f"se{mi}", bufs=1, name=f"semat{mi}")
            for mi, msz in enumerate(MSZ)
        ]

        # --- PE warm-up on a gpsimd-memset tile (no DMA dependency).
        wsrc = sb.tile([128, N], BF16, tag="warm", bufs=1, name="warmsrc")
        nc.gpsimd.memset(wsrc[:], 0.0)
        wps = ps.tile([1, N], F32, tag="wps", bufs=1, name="warmps")
        for i in range(N_WARM):
            nc.tensor.matmul(
                wps[:], ones_col, wsrc[:], start=(i == 0), stop=(i == N_WARM - 1)
            )

        def norm_chain(g, x):
            # Squares of the full [128, 4096] x tile, split across engines,
            # into one sq tile; 3-level pairwise add tree down to [128, 512];
            # partition_all_reduce -> n2 broadcast on all partitions;
            # rn = Exp(-0.5 * Ln(n2)).
            sq = sb.tile([128, XW], BF16, tag="sq", bufs=2, name=f"sq{g}")
            h = XW // 4  # 1024
            nc.vector.tensor_mul(sq[:, 0:h], x[:, 0:h], x[:, 0:h])
            nc.gpsimd.tensor_mul(sq[:, h : 2 * h], x[:, h : 2 * h], x[:, h : 2 * h])
            nc.scalar.square(sq[:, 2 * h : 4 * h], x[:, 2 * h : 4 * h])
            t2 = sb.tile([128, XW // 2], BF16, tag="t2", bufs=2, name=f"t2{g}")
            nc.vector.tensor_add(t2[:], sq[:, : XW // 2], sq[:, XW // 2 :])
            t1 = sb.tile([128, XW // 4], BF16, tag="t1", bufs=2, name=f"t1{g}")
            nc.vector.tensor_add(t1[:], t2[:, : XW // 4], t2[:, XW // 4 :])
            ssq = sb.tile([128, N], BF16, tag="ssq", bufs=2, name=f"ssq{g}")
            nc.vector.tensor_add(ssq[:], t1[:, :N], t1[:, N:])
            n2 = sb.tile([128, N], F32, tag="n2", bufs=2, name=f"n2{g}")
            nc.gpsimd.partition_all_reduce(
                n2[:], ssq[:], channels=128, reduce_op=bass_isa.ReduceOp.add
            )
            lnt = sb.tile([128, N], F32, tag="lnt", bufs=2, name=f"lnt{g}")
            nc.scalar.activation(lnt[:], n2[:], LN)
            rn = sb.tile([128, N], F32, tag="rn", bufs=3, name=f"rn{g}")
            nc.scalar.activation(rn[:], lnt[:], EXP, scale=-0.5)
            return rn

        def mm_chunk(g, x, coff, msz, nm):
            a = ps.tile([msz, N], F32, tag="sps", bufs=7, name=f"ps{nm}g{g}")
            for k in range(KC):
                nc.tensor.matmul(
                    a[:],
                    qpt_sb[:, k * 2 * I + coff : k * 2 * I + coff + msz],
                    x[:, k * N : (k + 1) * N],
                    start=(k == 0),
                    stop=(k == KC - 1),
                )
            return a

        def softmax_dot(g, mi, sqs, sps, msz):
            # sqs: scaled Q-side logits [msz, N]; sps: scaled P-side [msz, N].
            E = sb.tile([msz, N], F32, tag="E", bufs=2, name=f"Eg{g}m{mi}")
            for h in range(2):
                nc.scalar.activation(
                    E[:, h * R : (h + 1) * R],
                    sqs[:, h * R : (h + 1) * R],
                    EXP,
                    accum_out=semat[mi][:msz, 2 * g + h : 2 * g + h + 1],
                )
            scr = sb.tile([msz, R], F32, tag="scr", bufs=2, name=f"scrg{g}m{mi}")
            for h in range(2):
                nc.vector.scalar_tensor_tensor(
                    out=scr[:],
                    in0=E[:, h * R : (h + 1) * R],
                    scalar=1.0,
                    in1=sps[:, h * R : (h + 1) * R],
                    op0=MULT,
                    op1=MULT,
                    accum_out=outsb[mi][:msz, 2 * g + h : 2 * g + h + 1],
                )

        def main_group(g, x, rn):
            # Tail chunk first: its partition-shift DMA then overlaps the two
            # full chunk-pairs' drains instead of sitting at the group's end.
            ta = mm_chunk(g, x, MCH_T, 2 * TQ, "t")
            ts = sb.tile([2 * TQ, N], F32, tag="tss", bufs=2, name=f"tsg{g}")
            nc.vector.tensor_mul(ts[:], ta[:], rn[: 2 * TQ, :])
            tp = sb.tile([TQ, N], F32, tag="tps", bufs=2, name=f"tpg{g}")
            nc.sync.dma_start(tp[:, :], ts[TQ : 2 * TQ, :])
            for mi in range(2):
                qa = mm_chunk(g, x, MCH_Q[mi], 128, f"q{mi}")
                pa = mm_chunk(g, x, MCH_P[mi], 128, f"p{mi}")
                sqs = sb.tile([128, N], F32, tag="sqs", bufs=2, name=f"sqsg{g}m{mi}")
                nc.vector.tensor_mul(sqs[:], qa[:], rn[:, :])
                sps = sb.tile([128, N], F32, tag="spss", bufs=2, name=f"spsg{g}m{mi}")
                nc.vector.tensor_mul(sps[:], pa[:], rn[:, :])
                softmax_dot(g, mi, sqs, sps, 128)
            softmax_dot(g, 2, ts[:TQ, :], tp[:], TQ)

        def finalize(p):
            # Softmax denominators for the 4 batches of pair p + store.
            offs = [0, 128, 256]
            for mi, msz in enumerate(MSZ):
                cs = slice(4 * p, 4 * p + 4)
                rec = sb.tile([msz, 4], F32, tag=f"rec{mi}", bufs=2, name=f"rec{mi}p{p}")
                nc.vector.reciprocal(rec[:], semat[mi][:msz, cs])
                fin = sb.tile([msz, 4], F32, tag=f"fin{mi}", bufs=2, name=f"fin{mi}p{p}")
                nc.vector.tensor_mul(fin[:], outsb[mi][:msz, cs], rec[:])
                nc.sync.dma_start(out[offs[mi] : offs[mi] + msz, cs], fin[:])

        for g in range(G):
            rn = norm_chain(g, xs[g])
            main_group(g, xs.pop(g), rn)
            if g % 2 == 1:
                finalize(g // 2)

    nc.compile()
    return nc


def _prepare(inputs):
    img = np.asarray(inputs["img"], np.float32)
    V = np.asarray(inputs["V"], np.float32)
    W1 = np.asarray(inputs["W1"], np.float32)
    W2 = np.asarray(inputs["W2"], np.float32)
    B, Cf, H, W = img.shape
    assert (B, Cf, H * W) == (N_CORES * NB, CF, R), img.shape

    import ml_dtypes

    vv = V.astype(np.float64)
    vv /= np.maximum(np.sqrt((vv * vv).sum(1, keepdims=True)), 1e-12)
    Q = vv @ W1.astype(np.float64)  # [I, CF]
    P = vv @ W2.astype(np.float64)
    # Column order: Q[0:128], Q[128:256], P[0:128], P[128:256], Q[256:], P[256:]
    stacked = np.concatenate(
        [Q[0:128], Q[128:256], P[0:128], P[128:256], Q[256:I], P[256:I]], axis=0
    )
    qpt = np.ascontiguousarray(stacked.T.astype(ml_dtypes.bfloat16))  # [CF, 624]

    # Per-core img: [G, 128, KC * 2 * R] bf16 so each group is one big tile
    # whose k-th 512-column slice is [128 f-rows, 2 batches x 256 r] and the
    # DRAM lines are 8 KB per partition row.
    imgb = img.reshape(B, Cf, H * W).astype(ml_dtypes.bfloat16)
    imgb = imgb.reshape(N_CORES, G, 2, KC, 128, R).transpose(0, 1, 4, 3, 2, 5)
    imgb = np.ascontiguousarray(imgb.reshape(N_CORES, G, 128, KC * 2 * R))
    in_maps = [{"img": imgb[c], "qpt": qpt} for c in range(N_CORES)]
    return in_maps


def run(inputs, **spmd_kwargs):
    """Run the kernel; returns (full_output [B, I], BassKernelResults)."""
    global _PROGRAM
    if _PROGRAM is None:
        _PROGRAM = _build_program()
    from concourse.bass_utils import run_bass_kernel_spmd

    in_maps = _prepare(inputs)
    res = run_bass_kernel_spmd(
        _PROGRAM, in_maps, core_ids=list(range(N_CORES)), **spmd_kwargs
    )
    out = np.concatenate(
        [np.asarray(res.results[c]["out"]).T for c in range(N_CORES)], axis=0
    )
    return np.ascontiguousarray(out, np.float32), res


def kernel(**inputs) -> np.ndarray:
    return run(inputs)[0]
